# revision 1
# baseline (speedup 1.0000x reference)
"""Trainium2 Bass kernel for a 5-layer GPT-style transformer (BigramLanguageModel).

Sharding: data-parallel over batch (B=8 -> 1 sequence per core) through the
transformer layers (zero collectives), then AllGather of the final hidden
states and a vocab-parallel LM head (each core computes all 4096 tokens x its
4000-wide vocab shard).

Layout strategy per core:
  - residual stream h kept natural [T,D] (LN/softmax reduce over free dim)
  - LN outputs PE-transposed to [D,T] (f32r) for matmuls
  - attention scores computed TRANSPOSED ([s,t]); the causal mask is added to
    the PSUM scores (additive -1e9 on the diagonal block) BEFORE the exp so
    the attn@V matmul depends only on the exp; the softmax denominator comes
    from an appended ones-column in the V operand, reciprocated straight out
    of PSUM and applied via a ones[1,64] x recip-row broadcast matmul.
  - matmuls in float32r (full PE rate at N>=256, ~1e-4 relative rounding)
  - layer-phase SBUF pools close at the AllGather barrier, freeing ~116KB so
    the head holds the full 4000-wide Wout shard and stages full logit rows
    (single 2MB output DMA per 128 tokens, gathered activations read once)
  - LN gains==1 / biases==0 (checked at call time) skip their elementwise ops
"""

import sys

import numpy as np

sys.path.insert(0, "/opt/trn_rl_repo")

import concourse.bass as bass
import concourse.mybir as mybir
import concourse.tile as tile
from concourse import bacc
from concourse.bass_utils import run_bass_kernel_spmd

F32 = mybir.dt.float32
F32R = mybir.dt.float32r
I32 = mybir.dt.int32
AF = mybir.ActivationFunctionType
ALU = mybir.AluOpType

D, H, HS, L, V, CTX, B, T, FF = 384, 6, 64, 5, 32000, 512, 8, 512, 1536
P = 128
DT = D // P          # 3 d-tiles
TT = T // P          # 4 t-tiles
NT = FF // P         # 12 ff-tiles
N_CORES = 8
VSH = V // N_CORES   # 4000 vocab shard per core
VCH = 500            # vocab chunk per matmul (PSUM bank = 512 f32)
NCH = VSH // VCH     # 8 chunks per 128-token row
SCALE = float(D) ** -0.5
EPS = 1e-5


def _build(trivial_gb=False, trivial_bias=False, trivial_bout=False,
           sim_nocc=False):
    nc = bacc.Bacc("TRN2", target_bir_lowering=False, debug=False,
                   num_devices=1 if sim_nocc else N_CORES)

    io = {}
    io["x"] = nc.dram_tensor("x", [T], I32, kind="ExternalInput")
    io["tok_emb"] = nc.dram_tensor("tok_emb", [V, D], F32, kind="ExternalInput")
    io["pos_emb"] = nc.dram_tensor("pos_emb", [CTX, D], F32, kind="ExternalInput")
    for n, shp in [("ln1_g", [L, D]), ("ln1_b", [L, D]),
                   ("Wq", [L, D, D]), ("Wk", [L, D, D]), ("Wv", [L, D, D]),
                   ("Wproj", [L, D, D]), ("bproj", [L, D]),
                   ("ln2_g", [L, D]), ("ln2_b", [L, D]),
                   ("W1", [L, D, FF]), ("b1", [L, FF]),
                   ("W2", [L, FF, D]), ("b2", [L, D]),
                   ("lnf_g", [D]), ("lnf_b", [D]),
                   ("Wout_sh", [D, VSH]), ("bout_sh", [VSH])]:
        io[n] = nc.dram_tensor(n, shp, F32, kind="ExternalInput")
    io["logits_sh"] = nc.dram_tensor("logits_sh", [B * T, VSH], F32,
                                     kind="ExternalOutput")

    io["ident_d"] = nc.inline_tensor(np.eye(P, dtype=np.float32), name="ident_c")
    ntriu = (1.0 - np.triu(np.ones((P, P), np.float32))) * -1e9
    io["ntriu_d"] = nc.inline_tensor(ntriu.astype(np.float32), name="ntriu_c")
    io["ones64_d"] = nc.inline_tensor(np.ones((1, 64), np.float32),
                                      name="ones64_c")
    io["onesP_d"] = nc.inline_tensor(np.ones((P, 1), np.float32), name="onesP_c")

    with tile.TileContext(nc) as tc:
        _emit(nc, tc, io, trivial_gb, trivial_bias, trivial_bout, sim_nocc)
    nc.compile()
    return nc


def _emit(nc, tc, io, trivial_gb, trivial_bias, trivial_bout, sim_nocc):
    from contextlib import ExitStack
    octx = ExitStack()
    with octx:
        dram = octx.enter_context(tc.tile_pool(name="dram", bufs=1, space="DRAM"))
        pp_mm = octx.enter_context(tc.tile_pool(name="pp_mm", bufs=4,
                                                space="PSUM"))
        pp_tp = octx.enter_context(tc.tile_pool(name="pp_tp", bufs=2,
                                                space="PSUM"))
        pp_at = octx.enter_context(tc.tile_pool(name="pp_at", bufs=2,
                                                space="PSUM"))
        ag_in = dram.tile([D, T], F32, name="ag_in")
        VHALF = VSH // 2
        woutp = octx.enter_context(tc.tile_pool(name="woutp", bufs=1))
        wo_a = woutp.tile([P, DT, VHALF], F32R, name="wo_a", tag="wo_a")
        nc.sync.dma_start(
            out=wo_a[:],
            in_=io["Wout_sh"][:, 0:VHALF]
            .rearrange("(k p) n -> p k n", p=P).bitcast(F32R))

        with ExitStack() as ictx:
            const = ictx.enter_context(tc.tile_pool(name="const", bufs=1))
            hp = ictx.enter_context(tc.tile_pool(name="hp", bufs=1))
            act = ictx.enter_context(tc.tile_pool(name="act", bufs=6))
            atp = ictx.enter_context(tc.tile_pool(name="atp", bufs=2))
            qkp = ictx.enter_context(tc.tile_pool(name="qkp", bufs=1))
            vp = ictx.enter_context(tc.tile_pool(name="vp", bufs=5))
            ptp = ictx.enter_context(tc.tile_pool(name="ptp", bufs=8))
            up = ictx.enter_context(tc.tile_pool(name="up", bufs=3))
            otp = ictx.enter_context(tc.tile_pool(name="otp", bufs=1))
            gtp = ictx.enter_context(tc.tile_pool(name="gtp", bufs=2))
            wp = ictx.enter_context(tc.tile_pool(name="wp", bufs=6))
            w1p = ictx.enter_context(tc.tile_pool(name="w1p", bufs=1))
            w2p = ictx.enter_context(tc.tile_pool(name="w2p", bufs=1))
            bcp = ictx.enter_context(tc.tile_pool(name="bcp", bufs=4))
            smp = ictx.enter_context(tc.tile_pool(name="smp", bufs=4))

            ident = const.tile([P, P], F32, name="ident")
            nc.sync.dma_start(out=ident[:], in_=io["ident_d"][:])
            ntriu = const.tile([P, P], F32, name="ntriu")
            nc.sync.dma_start(out=ntriu[:], in_=io["ntriu_d"][:])
            ones64 = const.tile([1, 64], F32R, name="ones64")
            nc.sync.dma_start(out=ones64[:], in_=io["ones64_d"][:].bitcast(F32R))
            onesP = const.tile([P, 1], F32R, name="onesP")
            nc.sync.dma_start(out=onesP[:], in_=io["onesP_d"][:].bitcast(F32R))
            epsP = const.tile([P, 1], F32, name="epsP")
            nc.vector.memset(epsP[:], EPS)

            # ---- embedding gather + pos ----
            h_sb = []
            for m in range(TT):
                it = smp.tile([P, 1], I32, name=f"idx{m}", tag="idx")
                nc.sync.dma_start(out=it[:], in_=io["x"][P * m:P * (m + 1), None])
                ht = hp.tile([P, D], F32, name=f"h{m}", tag=f"h{m}")
                nc.gpsimd.indirect_dma_start(
                    out=ht[:], out_offset=None, in_=io["tok_emb"][:],
                    in_offset=bass.IndirectOffsetOnAxis(ap=it[:, :1], axis=0))
                pt = act.tile([P, D], F32, name=f"pos{m}", tag="af")
                nc.sync.dma_start(out=pt[:],
                                  in_=io["pos_emb"][P * m:P * (m + 1), :])
                nc.vector.tensor_tensor(out=ht[:], in0=ht[:], in1=pt[:],
                                        op=ALU.add)
                h_sb.append(ht)

            def layer_norm(src_tiles, g_dram, b_dram, tag):
                if not trivial_gb:
                    g_bc = bcp.tile([P, D], F32, name=f"g_{tag}", tag="gb")
                    nc.sync.dma_start(out=g_bc[:],
                                      in_=g_dram[None, :].to_broadcast([P, D]))
                    b_bc = bcp.tile([P, D], F32, name=f"b_{tag}", tag="gb")
                    nc.sync.dma_start(out=b_bc[:],
                                      in_=b_dram[None, :].to_broadcast([P, D]))
                outs = []
                for m in range(TT):
                    st = smp.tile([P, 6], F32, name=f"st_{tag}{m}", tag="st")
                    nc.vector.bn_stats(out=st[:], in_=src_tiles[m][:])
                    mv = smp.tile([P, 2], F32, name=f"mv_{tag}{m}", tag="mv")
                    nc.vector.bn_aggr(out=mv[:], in_=st[:])
                    nc.scalar.activation(out=mv[:, 1:2], in_=mv[:, 1:2],
                                         func=AF.Sqrt, bias=epsP[:])
                    nc.vector.reciprocal(out=mv[:, 1:2], in_=mv[:, 1:2])
                    at = act.tile([P, D], F32, name=f"a_{tag}{m}", tag="af")
                    nc.vector.tensor_scalar(out=at[:], in0=src_tiles[m][:],
                                            scalar1=mv[:, 0:1],
                                            scalar2=mv[:, 1:2],
                                            op0=ALU.subtract, op1=ALU.mult)
                    if not trivial_gb:
                        nc.vector.tensor_tensor(out=at[:], in0=at[:], in1=g_bc[:],
                                                op=ALU.mult)
                        nc.vector.tensor_tensor(out=at[:], in0=at[:], in1=b_bc[:],
                                                op=ALU.add)
                    outs.append(at)
                return outs

            def transpose_to(a_tiles, tag):
                """[TT][P,D] natural f32 -> [DT][P,T] f32r transposed"""
                outs = []
                for k in range(DT):
                    ps = pp_tp.tile([P, T], F32, name=f"tp_{tag}{k}", tag="tp")
                    for m in range(TT):
                        nc.tensor.transpose(ps[:, P * m:P * (m + 1)],
                                            a_tiles[m][:, P * k:P * (k + 1)],
                                            ident[:])
                    dst = atp.tile([P, T], F32R, name=f"{tag}T{k}", tag=f"aT{k}")
                    nc.scalar.copy(dst[:], ps[:])
                    outs.append(dst)
                return outs

            def load_w_dxd(w_dram, l, tag):
                """W[l] [D,D] -> one [P, DT, D] f32r tile (single DMA)"""
                wt = wp.tile([P, DT, D], F32R, name=f"{tag}{l}", tag="wdd")
                nc.sync.dma_start(
                    out=wt[:],
                    in_=w_dram[l].rearrange("(k p) n -> p k n", p=P).bitcast(F32R))
                return wt

            # ================= layers =================
            for l in range(L):
                a_t = layer_norm(h_sb, io["ln1_g"][l, :], io["ln1_b"][l, :],
                                 f"l{l}n1")
                aT = transpose_to(a_t, f"l{l}a")

                wq_t = load_w_dxd(io["Wq"], l, "wq")
                wk_t = load_w_dxd(io["Wk"], l, "wk")
                wv_t = load_w_dxd(io["Wv"], l, "wv")
                wpj_t = load_w_dxd(io["Wproj"], l, "wp")

                qT, kT = [], []
                for name, wt, dstl in (("q", wq_t, qT), ("k", wk_t, kT)):
                    for dq in range(DT):
                        ps = pp_mm.tile([P, T], F32, name=f"ps_{name}{l}{dq}",
                                        tag="mm")
                        for d in range(DT):
                            nc.tensor.matmul(ps[:], wt[:, d, P * dq:P * (dq + 1)],
                                             aT[d][:], start=(d == 0),
                                             stop=(d == DT - 1))
                        dst = qkp.tile([P, T], F32R, name=f"{name}T{l}{dq}",
                                       tag=f"{name}T{dq}")
                        nc.vector.tensor_copy(out=dst[:], in_=ps[:])
                        dstl.append(dst)
                v_sb = []
                for m in range(TT):
                    ps = pp_mm.tile([P, D], F32, name=f"ps_v{l}{m}", tag="mm")
                    for d in range(DT):
                        nc.tensor.matmul(ps[:], aT[d][:, P * m:P * (m + 1)],
                                         wv_t[:, d, :], start=(d == 0),
                                         stop=(d == DT - 1))
                    vt = vp.tile([P, H, HS + 1], F32R, name=f"v{l}{m}", tag="v")
                    nc.scalar.copy(vt[:, :, 0:HS],
                                   ps[:].rearrange("p (h d) -> p h d", h=H))
                    nc.vector.tensor_copy(out=vt[:, :, HS],
                                          in_=onesP[:, 0:1].to_broadcast([P, H]))
                    v_sb.append(vt)

                # attention per head -> oT [DT][P,T] f32r
                oT = [otp.tile([P, T], F32R, name=f"oT{l}{k}", tag=f"oT{k}")
                      for k in range(DT)]
                for h in range(H):
                    r, off = (h * HS) // P, (h * HS) % P
                    pT = []
                    for j in range(TT):
                        n_j = T - P * j
                        ps = pp_mm.tile([P, T], F32, name=f"ps_s{l}{h}{j}",
                                        tag="mm")
                        nc.tensor.matmul(
                            ps[:, 0:n_j],
                            kT[r][off:off + HS, P * j:P * (j + 1)],
                            qT[r][off:off + HS, P * j:T],
                            start=True, stop=True)
                        # causal mask: additive -1e9 on the diagonal block,
                        # applied in PSUM before the exp
                        nc.vector.tensor_tensor(out=ps[:, 0:P], in0=ps[:, 0:P],
                                                in1=ntriu[:], op=ALU.add)
                        pt = ptp.tile([P, T], F32R, name=f"pT{l}{h}{j}",
                                      tag="pT")
                        nc.scalar.activation(out=pt[:, 0:n_j], in_=ps[:, 0:n_j],
                                             func=AF.Exp, scale=SCALE)
                        pT.append(pt)
                    ups = pp_at.tile([HS + 1, T], F32, name=f"ups{l}{h}",
                                     tag="at")
                    for j in range(TT):
                        n_j = T - P * j
                        nc.tensor.matmul(ups[:, P * j:T], v_sb[j][:, h, :],
                                         pT[j][:, 0:n_j],
                                         start=(j == 0), stop=(j == TT - 1))
                    u = up.tile([HS, T], F32, name=f"u{l}{h}", tag="u")
                    nc.vector.tensor_copy(out=u[:], in_=ups[0:HS, :])
                    rec = smp.tile([1, T], F32R, name=f"rec{l}{h}", tag="rec")
                    with nc.allow_low_precision(reason="softmax denom in f32r"):
                        nc.vector.reciprocal(out=rec[:], in_=ups[HS:HS + 1, :])
                    bc = pp_at.tile([64, T], F32, name=f"bc{l}{h}", tag="at")
                    nc.tensor.matmul(bc[:], ones64[:], rec[:], start=True,
                                     stop=True)
                    nc.vector.tensor_tensor(out=oT[r][off:off + HS, :],
                                            in0=u[:], in1=bc[:], op=ALU.mult)

                # proj + residual (natural out)
                if not trivial_bias:
                    bp_bc = bcp.tile([P, D], F32, name=f"bp{l}", tag="gb")
                    nc.sync.dma_start(
                        out=bp_bc[:],
                        in_=io["bproj"][l, None, :].to_broadcast([P, D]))
                for m in range(TT):
                    ps = pp_mm.tile([P, D], F32, name=f"ps_pj{l}{m}", tag="mm")
                    for d in range(DT):
                        nc.tensor.matmul(ps[:], oT[d][:, P * m:P * (m + 1)],
                                         wpj_t[:, d, :], start=(d == 0),
                                         stop=(d == DT - 1))
                    if not trivial_bias:
                        nc.vector.tensor_tensor(out=ps[:], in0=ps[:],
                                                in1=bp_bc[:], op=ALU.add)
                    nc.vector.tensor_tensor(out=h_sb[m][:], in0=h_sb[m][:],
                                            in1=ps[:], op=ALU.add)

                # ---- FF ----
                f_t = layer_norm(h_sb, io["ln2_g"][l, :], io["ln2_b"][l, :],
                                 f"l{l}n2")
                fT = transpose_to(f_t, f"l{l}f")

                w1_t = w1p.tile([P, DT, FF], F32R, name=f"w1_{l}", tag="w1")
                nc.sync.dma_start(
                    out=w1_t[:],
                    in_=io["W1"][l].rearrange("(k p) n -> p k n",
                                              p=P).bitcast(F32R))
                b1c = smp.tile([P, NT], F32, name=f"b1c{l}", tag="b1c")
                nc.sync.dma_start(
                    out=b1c[:], in_=io["b1"][l, :].rearrange("(n p) -> p n", p=P))
                w2_t = w2p.tile([P, NT, D], F32R, name=f"w2_{l}", tag="w2")
                nc.sync.dma_start(
                    out=w2_t[:],
                    in_=io["W2"][l].rearrange("(k p) n -> p k n",
                                              p=P).bitcast(F32R))

                ps_h = [pp_mm.tile([P, D], F32, name=f"ps_ff{l}{m}", tag="mm")
                        for m in range(TT)]
                for nt in range(NT):
                    psg = pp_tp.tile([P, T], F32, name=f"ps_g{l}{nt}", tag="tp")
                    for d in range(DT):
                        nc.tensor.matmul(psg[:], w1_t[:, d, P * nt:P * (nt + 1)],
                                         fT[d][:], start=(d == 0),
                                         stop=(d == DT - 1))
                    gt = gtp.tile([P, T], F32R, name=f"g{l}{nt}", tag="g")
                    nc.scalar.activation(out=gt[:], in_=psg[:], func=AF.Relu,
                                         bias=b1c[:, nt:nt + 1])
                    for m in range(TT):
                        nc.tensor.matmul(ps_h[m][:], gt[:, P * m:P * (m + 1)],
                                         w2_t[:, nt, :], start=(nt == 0),
                                         stop=(nt == NT - 1))
                if not trivial_bias:
                    b2_bc = bcp.tile([P, D], F32, name=f"b2{l}", tag="gb")
                    nc.sync.dma_start(
                        out=b2_bc[:],
                        in_=io["b2"][l, None, :].to_broadcast([P, D]))
                for m in range(TT):
                    if not trivial_bias:
                        nc.vector.tensor_tensor(out=ps_h[m][:], in0=ps_h[m][:],
                                                in1=b2_bc[:], op=ALU.add)
                    nc.vector.tensor_tensor(out=h_sb[m][:], in0=h_sb[m][:],
                                            in1=ps_h[m][:], op=ALU.add)

            # ---- final LN -> transposed -> DRAM bounce ----
            hf_t = layer_norm(h_sb, io["lnf_g"][:], io["lnf_b"][:], "lnf")
            hfT = transpose_to(hf_t, "hf")
            for k in range(DT):
                nc.sync.dma_start(out=ag_in[P * k:P * (k + 1), :],
                                  in_=hfT[k][:].bitcast(F32))
        # layer-phase SBUF pools closed here (AllGather is the barrier anyway)

        if sim_nocc:
            ag_out = dram.tile([N_CORES * D, T], F32, name="ag_out")
            for rr in range(N_CORES):
                nc.sync.dma_start(out=ag_out[rr * D:(rr + 1) * D, :],
                                  in_=ag_in[:])
        else:
            ag_out = dram.tile([N_CORES * D, T], F32, name="ag_out",
                               addr_space="Shared")
            nc.gpsimd.collective_compute(
                "AllGather", ALU.bypass,
                replica_groups=[list(range(N_CORES))],
                ins=[ag_in[:].opt()], outs=[ag_out[:].opt()])

        # ================= vocab-parallel head (full width) =================
        hfp = octx.enter_context(tc.tile_pool(name="hfp", bufs=4))
        lop = octx.enter_context(tc.tile_pool(name="lop", bufs=3))
        boutp = octx.enter_context(tc.tile_pool(name="boutp", bufs=1))

        wo_b = woutp.tile([P, DT, VHALF], F32R, name="wo_b", tag="wo_b")
        nc.sync.dma_start(
            out=wo_b[:],
            in_=io["Wout_sh"][:, VHALF:]
            .rearrange("(k p) n -> p k n", p=P).bitcast(F32R))

        def wo_slice(d, nb):
            c0 = VCH * nb
            if c0 < VHALF:
                return wo_a[:, d, c0:c0 + VCH]
            return wo_b[:, d, c0 - VHALF:c0 - VHALF + VCH]
        if not trivial_bout:
            bo_bc = boutp.tile([P, VSH], F32, name="bo", tag="bo")
            nc.sync.dma_start(
                out=bo_bc[:], in_=io["bout_sh"][None, :].to_broadcast([P, VSH]))
        ps_pools = [pp_mm] * 4 + [pp_tp] * 2 + [pp_at] * 2
        ps_tags = ["mm"] * 4 + ["tp"] * 2 + ["at"] * 2
        for b in range(N_CORES):
            hb = hfp.tile([P, DT, T], F32R, name=f"hf{b}", tag="hf")
            nc.sync.dma_start(
                out=hb[:],
                in_=ag_out[b * D:(b + 1) * D, :]
                .rearrange("(k p) n -> p k n", p=P).bitcast(F32R))
            for m in range(TT):
                row0 = b * T + P * m
                lo = lop.tile([P, VSH], F32, name=f"lo{b}{m}", tag="lo")
                for nb in range(NCH):
                    ps = ps_pools[nb].tile([P, VCH], F32,
                                           name=f"ps_o{b}{m}{nb}",
                                           tag=ps_tags[nb])
                    for d in range(DT):
                        nc.tensor.matmul(ps[:], hb[:, d, P * m:P * (m + 1)],
                                         wo_slice(d, nb),
                                         start=(d == 0), stop=(d == DT - 1))
                    sl = lo[:, VCH * nb:VCH * (nb + 1)]
                    if trivial_bout:
                        # alternate eviction engine: ACT and DVE each take half
                        if nb % 2 == 0:
                            nc.scalar.copy(sl, ps[:])
                        else:
                            nc.vector.tensor_copy(out=sl, in_=ps[:])
                    else:
                        nc.vector.tensor_tensor(
                            out=sl, in0=ps[:],
                            in1=bo_bc[:, VCH * nb:VCH * (nb + 1)], op=ALU.add)
                nc.sync.dma_start(out=io["logits_sh"][row0:row0 + P, :],
                                  in_=lo[:])


_NC_CACHE = {}


def _get_nc(trivial_gb=False, trivial_bias=False, trivial_bout=False):
    key = (trivial_gb, trivial_bias, trivial_bout)
    if key not in _NC_CACHE:
        _NC_CACHE[key] = _build(*key)
    return _NC_CACHE[key]


def _build_sim():
    return _build(trivial_gb=True, trivial_bias=True, trivial_bout=True,
                  sim_nocc=True)


def kernel(**inputs):
    inp = {k: np.ascontiguousarray(np.asarray(v)) for k, v in inputs.items()}
    trivial_gb = all(
        np.all(inp[g] == 1.0) and np.all(inp[b] == 0.0)
        for g, b in [("ln1_g", "ln1_b"), ("ln2_g", "ln2_b"), ("lnf_g", "lnf_b")])
    trivial_bias = all(np.all(inp[b] == 0.0) for b in ("bproj", "b2"))
    trivial_bout = bool(np.all(inp["bout"] == 0.0))
    nc = _get_nc(trivial_gb, trivial_bias, trivial_bout)
    in_maps = []
    for c in range(N_CORES):
        m = {
            "x": inp["x"][c].astype(np.int32),
            "tok_emb": inp["tok_emb"], "pos_emb": inp["pos_emb"],
            "ln1_g": inp["ln1_g"], "ln1_b": inp["ln1_b"],
            "Wq": inp["Wq"], "Wk": inp["Wk"], "Wv": inp["Wv"],
            "Wproj": inp["Wproj"], "bproj": inp["bproj"],
            "ln2_g": inp["ln2_g"], "ln2_b": inp["ln2_b"],
            "W1": inp["W1"], "b1": inp["b1"], "W2": inp["W2"], "b2": inp["b2"],
            "lnf_g": inp["lnf_g"], "lnf_b": inp["lnf_b"],
            "Wout_sh": np.ascontiguousarray(inp["Wout"][:, c * VSH:(c + 1) * VSH]),
            "bout_sh": np.ascontiguousarray(inp["bout"][c * VSH:(c + 1) * VSH]),
        }
        in_maps.append(m)
    res = run_bass_kernel_spmd(nc, in_maps, core_ids=list(range(N_CORES)))
    parts = [res.results[c]["logits_sh"].reshape(B, T, VSH) for c in range(N_CORES)]
    return np.concatenate(parts, axis=2)



# revision 9
# speedup vs baseline: 1.2456x; 1.2456x over previous
"""Trainium2 Bass kernel for a 5-layer GPT-style transformer (BigramLanguageModel).

Sharding: pure data-parallel — each of the 8 cores runs one full sequence
through the transformer AND its own full-vocab LM head (token-sharded head).
Zero collectives; per-core output is [T, V] fp16, upcast + stacked on host.

Per-core layout strategy:
  - residual stream h kept natural [T,D] f32 (LN/softmax reduce over free dim)
  - all matmul operands in bf16 (full PE rate at any free size); weights are
    pre-cast to bf16 on the host, activations rounded at PSUM eviction
  - LN outputs PE-transposed to [D,T] bf16 for the QKV/FF matmuls
  - attention scores computed TRANSPOSED ([s,t]); the causal mask is a
    -1e9 upper-triangle ADDED IN PSUM by an extra identity matmul (PE work
    instead of a DVE pass), then exp on ACT straight out of PSUM -> bf16
  - softmax denominator comes from an appended ones-column in V; the
    reciprocal row is broadcast to 2 heads at a time with a single K=2
    matmul (f32r at N=512 streams at full rate)
  - LM head: full-vocab, streamed in 2000-wide bf16 Wout chunks reused
    across the 4 token tiles; logits staged fp16 (halves the output DMA,
    the dominant HBM cost) and upcast to f32 on the host
  - LN gains==1 / biases==0 (checked at call time) skip their elementwise ops
"""

import sys

import numpy as np

sys.path.insert(0, "/opt/trn_rl_repo")

import ml_dtypes

import concourse.bass as bass
import concourse.mybir as mybir
import concourse.tile as tile
from concourse import bacc
from concourse.bass_utils import run_bass_kernel_spmd

F32 = mybir.dt.float32
F32R = mybir.dt.float32r
BF16 = mybir.dt.bfloat16
F16 = mybir.dt.float16
I32 = mybir.dt.int32
AF = mybir.ActivationFunctionType
ALU = mybir.AluOpType

D, H, HS, L, V, CTX, B, T, FF = 384, 6, 64, 5, 32000, 512, 8, 512, 1536
P = 128
DT = D // P          # 3 d-tiles
TT = T // P          # 4 t-tiles
NT = FF // P         # 12 ff-tiles
N_CORES = 8
VC = 2000            # vocab chunk streamed per Wout load
NVC = V // VC        # 16 chunks
SUB = 500            # PSUM sub-chunk of a vocab chunk (bank = 512 f32)
NSUB = VC // SUB     # 4
SCALE = float(D) ** -0.5
EPS = 1e-5

BF16_NP = ml_dtypes.bfloat16


def _build(trivial_gb=False, trivial_bias=False):
    nc = bacc.Bacc("TRN2", target_bir_lowering=False, debug=False,
                   num_devices=1)

    io = {}
    io["x"] = nc.dram_tensor("x", [T], I32, kind="ExternalInput")
    io["tok_emb"] = nc.dram_tensor("tok_emb", [V, D], F32, kind="ExternalInput")
    io["pos_emb"] = nc.dram_tensor("pos_emb", [CTX, D], F32, kind="ExternalInput")
    for n, shp in [("ln1_g", [L, D]), ("ln1_b", [L, D]),
                   ("bproj", [L, D]),
                   ("ln2_g", [L, D]), ("ln2_b", [L, D]),
                   ("b1", [L, FF]), ("b2", [L, D]),
                   ("lnf_g", [D]), ("lnf_b", [D])]:
        io[n] = nc.dram_tensor(n, shp, F32, kind="ExternalInput")
    for n, shp in [("Wq", [L, D, D]), ("Wk", [L, D, D]), ("Wv", [L, D, D]),
                   ("Wproj", [L, D, D]),
                   ("W1", [L, D, FF]), ("W2", [L, FF, D]),
                   ("Wout", [D, V])]:
        io[n] = nc.dram_tensor(n, shp, BF16, kind="ExternalInput")
    io["logits"] = nc.dram_tensor("logits", [T, V], F16, kind="ExternalOutput")

    io["ident_d"] = nc.inline_tensor(np.eye(P, dtype=BF16_NP), name="ident_c")
    ntriu = ((1.0 - np.triu(np.ones((P, P), np.float32))) * -1e9)
    io["ntriu_d"] = nc.inline_tensor(ntriu.astype(BF16_NP), name="ntriu_c")
    io["ones64_d"] = nc.inline_tensor(np.ones((1, HS), np.float32),
                                      name="ones64_c")

    with tile.TileContext(nc) as tc:
        _emit(nc, tc, io, trivial_gb, trivial_bias)
    nc.compile()
    return nc


def _emit(nc, tc, io, trivial_gb, trivial_bias):
    from contextlib import ExitStack
    with ExitStack() as ictx:
        pp_mm = ictx.enter_context(tc.tile_pool(name="pp_mm", bufs=4,
                                                space="PSUM"))
        pp_tp = ictx.enter_context(tc.tile_pool(name="pp_tp", bufs=2,
                                                space="PSUM"))
        pp_at = ictx.enter_context(tc.tile_pool(name="pp_at", bufs=2,
                                                space="PSUM"))
        const = ictx.enter_context(tc.tile_pool(name="const", bufs=1))
        hp = ictx.enter_context(tc.tile_pool(name="hp", bufs=1))
        act = ictx.enter_context(tc.tile_pool(name="act", bufs=6))
        atp = ictx.enter_context(tc.tile_pool(name="atp", bufs=2))
        qkp = ictx.enter_context(tc.tile_pool(name="qkp", bufs=1))
        vp = ictx.enter_context(tc.tile_pool(name="vp", bufs=5))
        ptp = ictx.enter_context(tc.tile_pool(name="ptp", bufs=8))
        up = ictx.enter_context(tc.tile_pool(name="up", bufs=2))
        otp = ictx.enter_context(tc.tile_pool(name="otp", bufs=1))
        gtp = ictx.enter_context(tc.tile_pool(name="gtp", bufs=3))
        wp = ictx.enter_context(tc.tile_pool(name="wp", bufs=8))
        w1p = ictx.enter_context(tc.tile_pool(name="w1p", bufs=2))
        w2p = ictx.enter_context(tc.tile_pool(name="w2p", bufs=2))
        bcp = ictx.enter_context(tc.tile_pool(name="bcp", bufs=4))
        smp = ictx.enter_context(tc.tile_pool(name="smp", bufs=4))
        hfp = ictx.enter_context(tc.tile_pool(name="hfp", bufs=1))
        wop = ictx.enter_context(tc.tile_pool(name="wop", bufs=4))
        lop = ictx.enter_context(tc.tile_pool(name="lop", bufs=4))

        # ---- constants + embedding inputs first on the sync DMA queue ----
        epsP = const.tile([P, 1], F32, name="epsP")
        nc.vector.memset(epsP[:], EPS)
        ident = const.tile([P, P], BF16, name="ident")
        nc.sync.dma_start(out=ident[:], in_=io["ident_d"][:])
        ntriu = const.tile([P, P], BF16, name="ntriu")
        nc.sync.dma_start(out=ntriu[:], in_=io["ntriu_d"][:])
        ones64 = const.tile([1, HS], F32R, name="ones64")
        nc.sync.dma_start(out=ones64[:], in_=io["ones64_d"][:].bitcast(F32R))

        h_sb = []
        for m in range(TT):
            it = smp.tile([P, 1], I32, name=f"idx{m}", tag="idx")
            nc.sync.dma_start(out=it[:], in_=io["x"][P * m:P * (m + 1), None])
            ht = hp.tile([P, D], F32, name=f"h{m}", tag=f"h{m}")
            nc.gpsimd.indirect_dma_start(
                out=ht[:], out_offset=None, in_=io["tok_emb"][:],
                in_offset=bass.IndirectOffsetOnAxis(ap=it[:, :1], axis=0))
            pt = act.tile([P, D], F32, name=f"pos{m}", tag="af")
            nc.sync.dma_start(out=pt[:],
                              in_=io["pos_emb"][P * m:P * (m + 1), :])
            nc.gpsimd.tensor_tensor(out=ht[:], in0=ht[:], in1=pt[:],
                                    op=ALU.add)
            h_sb.append(ht)

        # ---- weight prefetch helpers ----
        def load_w_dxd(w_dram, l, tag):
            wt = wp.tile([P, DT, D], BF16, name=f"{tag}{l}", tag="wdd")
            nc.sync.dma_start(
                out=wt[:], in_=w_dram[l].rearrange("(k p) n -> p k n", p=P))
            return wt

        wts = {}

        def load_layer_weights(l):
            wq = load_w_dxd(io["Wq"], l, "wq")
            wk = load_w_dxd(io["Wk"], l, "wk")
            wv = load_w_dxd(io["Wv"], l, "wv")
            wpj = load_w_dxd(io["Wproj"], l, "wp")
            w1 = w1p.tile([P, DT, FF], BF16, name=f"w1_{l}", tag="w1")
            nc.sync.dma_start(
                out=w1[:], in_=io["W1"][l].rearrange("(k p) n -> p k n", p=P))
            w2 = w2p.tile([P, NT, D], BF16, name=f"w2_{l}", tag="w2")
            nc.sync.dma_start(
                out=w2[:], in_=io["W2"][l].rearrange("(k p) n -> p k n", p=P))
            wts[l] = (wq, wk, wv, wpj, w1, w2)

        def layer_norm(src_tiles, g_dram, b_dram, tag):
            if not trivial_gb:
                g_bc = bcp.tile([P, D], F32, name=f"g_{tag}", tag="gb")
                nc.sync.dma_start(out=g_bc[:],
                                  in_=g_dram[None, :].to_broadcast([P, D]))
                b_bc = bcp.tile([P, D], F32, name=f"b_{tag}", tag="gb")
                nc.sync.dma_start(out=b_bc[:],
                                  in_=b_dram[None, :].to_broadcast([P, D]))
            outs = []
            for m in range(TT):
                st = smp.tile([P, 6], F32, name=f"st_{tag}{m}", tag="st")
                nc.vector.bn_stats(out=st[:], in_=src_tiles[m][:])
                mv = smp.tile([P, 2], F32, name=f"mv_{tag}{m}", tag="mv")
                nc.vector.bn_aggr(out=mv[:], in_=st[:])
                nc.scalar.activation(out=mv[:, 1:2], in_=mv[:, 1:2],
                                     func=AF.Sqrt, bias=epsP[:])
                nc.vector.reciprocal(out=mv[:, 1:2], in_=mv[:, 1:2])
                at = act.tile([P, D], BF16, name=f"a_{tag}{m}", tag="af")
                with nc.allow_low_precision(reason="bf16 matmul operands"):
                    nc.vector.tensor_scalar(out=at[:], in0=src_tiles[m][:],
                                            scalar1=mv[:, 0:1],
                                            scalar2=mv[:, 1:2],
                                            op0=ALU.subtract, op1=ALU.mult)
                    if not trivial_gb:
                        nc.vector.tensor_tensor(out=at[:], in0=at[:],
                                                in1=g_bc[:], op=ALU.mult)
                        nc.vector.tensor_tensor(out=at[:], in0=at[:],
                                                in1=b_bc[:], op=ALU.add)
                outs.append(at)
            return outs

        def transpose_to(a_tiles, tag, pool, evict):
            """[TT][P,D] natural bf16 -> [DT][P,T] bf16 transposed"""
            outs = []
            for k in range(DT):
                ps = pp_tp.tile([P, T], BF16, name=f"tp_{tag}{k}", tag="tp")
                for m in range(TT):
                    nc.tensor.transpose(ps[:, P * m:P * (m + 1)],
                                        a_tiles[m][:, P * k:P * (k + 1)],
                                        ident[:])
                dst = pool.tile([P, T], BF16, name=f"{tag}T{k}", tag=f"aT{k}")
                evict(dst, ps)
                outs.append(dst)
            return outs

        def ev_act(dst, ps):
            nc.scalar.copy(dst[:], ps[:])

        def ev_dve(dst, ps):
            nc.vector.tensor_copy(out=dst[:], in_=ps[:])

        # ================= layers =================
        load_layer_weights(0)
        for l in range(L):
            if l + 1 < L:
                load_layer_weights(l + 1)
            wq_t, wk_t, wv_t, wpj_t, w1_t, w2_t = wts.pop(l)
            if not trivial_bias:
                b1c = smp.tile([P, NT], F32, name=f"b1c{l}", tag="b1c")
                nc.sync.dma_start(
                    out=b1c[:],
                    in_=io["b1"][l, :].rearrange("(n p) -> p n", p=P))

            a_t = layer_norm(h_sb, io["ln1_g"][l, :], io["ln1_b"][l, :],
                             f"l{l}n1")
            aT = transpose_to(a_t, f"l{l}a", atp, ev_act)

            qT, kT = [], []
            for name, wt, dstl in (("q", wq_t, qT), ("k", wk_t, kT)):
                for dq in range(DT):
                    ps = pp_mm.tile([P, T], F32, name=f"ps_{name}{l}{dq}",
                                    tag="mm")
                    for d in range(DT):
                        nc.tensor.matmul(ps[:], wt[:, d, P * dq:P * (dq + 1)],
                                         aT[d][:], start=(d == 0),
                                         stop=(d == DT - 1))
                    dst = qkp.tile([P, T], BF16, name=f"{name}T{l}{dq}",
                                   tag=f"{name}T{dq}")
                    nc.vector.tensor_copy(out=dst[:], in_=ps[:])
                    dstl.append(dst)
            v_sb = []
            for m in range(TT):
                ps = pp_mm.tile([P, D], F32, name=f"ps_v{l}{m}", tag="mm")
                for d in range(DT):
                    nc.tensor.matmul(ps[:], aT[d][:, P * m:P * (m + 1)],
                                     wv_t[:, d, :], start=(d == 0),
                                     stop=(d == DT - 1))
                vt = vp.tile([P, H, HS + 1], BF16, name=f"v{l}{m}", tag="v")
                nc.scalar.copy(vt[:, :, 0:HS],
                               ps[:].rearrange("p (h d) -> p h d", h=H))
                nc.gpsimd.memset(vt[:, :, HS], 1.0)
                v_sb.append(vt)

            # attention, 2 heads (one oT row-block) at a time
            oT = [otp.tile([P, T], BF16, name=f"oT{l}{k}", tag=f"oT{k}")
                  for k in range(DT)]
            for r in range(DT):
                upss = []
                for hh in range(2):
                    h = 2 * r + hh
                    off = hh * HS
                    pT = []
                    for j in range(TT):
                        n_j = T - P * j
                        ps = pp_mm.tile([P, T], F32, name=f"ps_s{l}{h}{j}",
                                        tag="mm")
                        nc.tensor.matmul(
                            ps[:, 0:n_j],
                            kT[r][off:off + HS, P * j:P * (j + 1)],
                            qT[r][off:off + HS, P * j:T],
                            start=True, stop=False, skip_group_check=True)
                        # causal mask: -1e9 upper triangle accumulated onto
                        # the diagonal block by the PE (not a DVE pass)
                        nc.tensor.matmul(
                            ps[:, 0:P], ident[:], ntriu[:],
                            start=False, stop=True, skip_group_check=True)
                        pt = ptp.tile([P, T], BF16, name=f"pT{l}{h}{j}",
                                      tag="pT")
                        nc.scalar.activation(out=pt[:, 0:n_j], in_=ps[:, 0:n_j],
                                             func=AF.Exp, scale=SCALE)
                        pT.append(pt)
                    ups = pp_at.tile([HS + 1, T], F32, name=f"ups{l}{h}",
                                     tag="at")
                    for j in range(TT):
                        n_j = T - P * j
                        nc.tensor.matmul(ups[:, P * j:T], v_sb[j][:, h, :],
                                         pT[j][:, 0:n_j],
                                         start=(j == 0), stop=(j == TT - 1))
                    upss.append(ups)
                with nc.allow_low_precision(reason="softmax denom/attn bf16"):
                    for hh in range(2):
                        off = hh * HS
                        u = up.tile([HS, T], BF16, name=f"u{l}{r}{hh}",
                                    tag="u")
                        nc.vector.tensor_copy(out=u[:],
                                              in_=upss[hh][0:HS, :])
                        rec = smp.tile([1, T], F32R, name=f"rec{l}{r}{hh}",
                                       tag="rec")
                        nc.vector.reciprocal(out=rec[:],
                                             in_=upss[hh][HS:HS + 1, :])
                        bc = pp_at.tile([HS, T], F32, name=f"bc{l}{r}{hh}",
                                        tag="at")
                        nc.tensor.matmul(bc[:], ones64[:], rec[:], start=True,
                                         stop=True)
                        nc.vector.tensor_tensor(out=oT[r][off:off + HS, :],
                                                in0=u[:], in1=bc[:],
                                                op=ALU.mult)

            # proj + residual (natural out)
            if not trivial_bias:
                bp_bc = bcp.tile([P, D], F32, name=f"bp{l}", tag="gb")
                nc.sync.dma_start(
                    out=bp_bc[:],
                    in_=io["bproj"][l, None, :].to_broadcast([P, D]))
            for m in range(TT):
                ps = pp_mm.tile([P, D], F32, name=f"ps_pj{l}{m}", tag="mm")
                for d in range(DT):
                    nc.tensor.matmul(ps[:], oT[d][:, P * m:P * (m + 1)],
                                     wpj_t[:, d, :], start=(d == 0),
                                     stop=(d == DT - 1))
                if not trivial_bias:
                    nc.vector.tensor_tensor(out=ps[:], in0=ps[:],
                                            in1=bp_bc[:], op=ALU.add)
                nc.vector.tensor_tensor(out=h_sb[m][:], in0=h_sb[m][:],
                                        in1=ps[:], op=ALU.add)

            # ---- FF ----
            f_t = layer_norm(h_sb, io["ln2_g"][l, :], io["ln2_b"][l, :],
                             f"l{l}n2")
            fT = transpose_to(f_t, f"l{l}f", atp, ev_act)

            ps_h = [pp_mm.tile([P, D], F32, name=f"ps_ff{l}{m}", tag="mm")
                    for m in range(TT)]
            for nt in range(NT):
                psg = pp_tp.tile([P, T], F32, name=f"ps_g{l}{nt}", tag="tp")
                for d in range(DT):
                    nc.tensor.matmul(psg[:], w1_t[:, d, P * nt:P * (nt + 1)],
                                     fT[d][:], start=(d == 0),
                                     stop=(d == DT - 1))
                gt = gtp.tile([P, T], BF16, name=f"g{l}{nt}", tag="g")
                if trivial_bias:
                    nc.scalar.activation(out=gt[:], in_=psg[:], func=AF.Relu)
                else:
                    nc.scalar.activation(out=gt[:], in_=psg[:], func=AF.Relu,
                                         bias=b1c[:, nt:nt + 1])
                for m in range(TT):
                    nc.tensor.matmul(ps_h[m][:], gt[:, P * m:P * (m + 1)],
                                     w2_t[:, nt, :], start=(nt == 0),
                                     stop=(nt == NT - 1))
            if not trivial_bias:
                b2_bc = bcp.tile([P, D], F32, name=f"b2{l}", tag="gb")
                nc.sync.dma_start(
                    out=b2_bc[:],
                    in_=io["b2"][l, None, :].to_broadcast([P, D]))
            for m in range(TT):
                if not trivial_bias:
                    nc.vector.tensor_tensor(out=ps_h[m][:], in0=ps_h[m][:],
                                            in1=b2_bc[:], op=ALU.add)
                nc.vector.tensor_tensor(out=h_sb[m][:], in0=h_sb[m][:],
                                        in1=ps_h[m][:], op=ALU.add)

        # ---- final LN -> transposed, stays in SBUF for the head ----
        hf_t = layer_norm(h_sb, io["lnf_g"][:], io["lnf_b"][:], "lnf")
        hfT = transpose_to(hf_t, "hf", hfp, ev_dve)

        # ================= token-sharded full-vocab head =================
        wo_tiles = {}

        def load_wo(vc):
            wo = wop.tile([P, DT, VC], BF16, name=f"wo{vc}", tag="wo")
            nc.sync.dma_start(
                out=wo[:],
                in_=io["Wout"][:, vc * VC:(vc + 1) * VC]
                .rearrange("(k p) n -> p k n", p=P))
            wo_tiles[vc] = wo

        for vc in range(min(4, NVC)):
            load_wo(vc)

        ps_pools = [pp_mm] * 4 + [pp_tp] * 2 + [pp_at] * 2
        ps_tags = ["mm"] * 4 + ["tp"] * 2 + ["at"] * 2
        # eviction engine rotation: ACT and DVE alternate
        for vc in range(NVC):
            wo = wo_tiles.pop(vc)
            if vc + 4 < NVC:
                load_wo(vc + 4)
            for m in range(TT):
                lo = lop.tile([P, VC], F16, name=f"lo{vc}{m}", tag="lo")
                for sub in range(NSUB):
                    k8 = (m * NSUB + sub) % 8
                    ps = ps_pools[k8].tile([P, SUB], F32,
                                           name=f"ps_o{vc}{m}{sub}",
                                           tag=ps_tags[k8])
                    c0 = sub * SUB
                    for d in range(DT):
                        nc.tensor.matmul(ps[:], hfT[d][:, P * m:P * (m + 1)],
                                         wo[:, d, c0:c0 + SUB],
                                         start=(d == 0), stop=(d == DT - 1))
                    sl = lo[:, c0:c0 + SUB]
                    if sub % 2 == 0:
                        nc.scalar.copy(sl, ps[:])
                    else:
                        nc.vector.tensor_copy(out=sl, in_=ps[:])
                nc.sync.dma_start(
                    out=io["logits"][P * m:P * (m + 1),
                                     vc * VC:(vc + 1) * VC],
                    in_=lo[:])


_NC_CACHE = {}


def _get_nc(trivial_gb=False, trivial_bias=False):
    key = (trivial_gb, trivial_bias)
    if key not in _NC_CACHE:
        _NC_CACHE[key] = _build(*key)
    return _NC_CACHE[key]


def _build_sim():
    return _build(trivial_gb=True, trivial_bias=True)


def kernel(**inputs):
    inp = {k: np.ascontiguousarray(np.asarray(v)) for k, v in inputs.items()}
    trivial_gb = all(
        np.all(inp[g] == 1.0) and np.all(inp[b] == 0.0)
        for g, b in [("ln1_g", "ln1_b"), ("ln2_g", "ln2_b"), ("lnf_g", "lnf_b")])
    trivial_bias = all(np.all(inp[b] == 0.0) for b in ("bproj", "b1", "b2"))
    nc = _get_nc(trivial_gb, trivial_bias)
    bw = {k: inp[k].astype(BF16_NP)
          for k in ("Wq", "Wk", "Wv", "Wproj", "W1", "W2", "Wout")}
    in_maps = []
    for c in range(N_CORES):
        m = {
            "x": inp["x"][c].astype(np.int32),
            "tok_emb": inp["tok_emb"], "pos_emb": inp["pos_emb"],
            "ln1_g": inp["ln1_g"], "ln1_b": inp["ln1_b"],
            "bproj": inp["bproj"],
            "ln2_g": inp["ln2_g"], "ln2_b": inp["ln2_b"],
            "b1": inp["b1"], "b2": inp["b2"],
            "lnf_g": inp["lnf_g"], "lnf_b": inp["lnf_b"],
            **bw,
        }
        in_maps.append(m)
    res = run_bass_kernel_spmd(nc, in_maps, core_ids=list(range(N_CORES)))
    out = np.stack([res.results[c]["logits"].astype(np.float32)
                    for c in range(N_CORES)], axis=0)
    if np.any(inp["bout"] != 0.0):
        out = out + inp["bout"][None, None, :].astype(np.float32)
    return out


# revision 86
# speedup vs baseline: 1.3077x; 1.0499x over previous
"""Trainium2 Bass kernel for a 5-layer GPT-style transformer (BigramLanguageModel).

Sharding: pure data-parallel — each of the 8 cores runs one full sequence
through the transformer AND its own full-vocab LM head (token-sharded head).
Zero collectives; per-core output is [T, V] fp16, upcast + stacked on host.

Per-core layout strategy:
  - residual stream h kept natural [T,D] f32 (LN/softmax reduce over free dim)
  - all matmul operands in bf16 (full PE rate at any free size); weights are
    pre-cast to bf16 on the host, activations rounded at PSUM eviction
  - LN outputs PE-transposed to [D,T] bf16 for the QKV/FF matmuls
  - attention scores computed TRANSPOSED ([s,t]); the causal mask is a
    -1e9 upper-triangle ADDED IN PSUM by an extra identity matmul (PE work
    instead of a DVE pass), then exp on ACT straight out of PSUM -> bf16
  - softmax denominator comes from an appended ones-column in V; the
    reciprocal row is broadcast to 2 heads at a time with a single K=2
    matmul (f32r at N=512 streams at full rate)
  - LM head: full-vocab, streamed in 2000-wide bf16 Wout chunks reused
    across the 4 token tiles; logits staged fp16 (halves the output DMA,
    the dominant HBM cost) and upcast to f32 on the host
  - LN gains==1 / biases==0 (checked at call time) skip their elementwise ops
"""

import sys

import numpy as np

sys.path.insert(0, "/opt/trn_rl_repo")

import ml_dtypes

import concourse.bass as bass
import concourse.mybir as mybir
import concourse.tile as tile
from concourse import bacc
from concourse.bass_utils import run_bass_kernel_spmd

F32 = mybir.dt.float32
F32R = mybir.dt.float32r
BF16 = mybir.dt.bfloat16
F16 = mybir.dt.float16
I32 = mybir.dt.int32
AF = mybir.ActivationFunctionType
ALU = mybir.AluOpType

D, H, HS, L, V, CTX, B, T, FF = 384, 6, 64, 5, 32000, 512, 8, 512, 1536
P = 128
DT = D // P          # 3 d-tiles
TT = T // P          # 4 t-tiles
NT = FF // P         # 12 ff-tiles
N_CORES = 8
VC = 1000            # vocab chunk streamed per Wout load
NVC = V // VC        # 32 chunks
SUB = 500            # PSUM sub-chunk of a vocab chunk (bank = 512 f32)
NSUB = VC // SUB     # 2
SCALE = float(D) ** -0.5
EPS = 1e-5

BF16_NP = ml_dtypes.bfloat16


DEBUG_TAPS = False


def _build(trivial_gb=False, trivial_bias=False):
    nc = bacc.Bacc("TRN2", target_bir_lowering=False, debug=False,
                   num_devices=1)

    io = {}
    if DEBUG_TAPS:
        for i in range(8):
            io[f"dbg{i}"] = nc.dram_tensor(f"dbg{i}", [P, TT, D], F32,
                                           kind="ExternalOutput")
        for n in ("dbgA", "dbgB", "dbgC", "dbgD", "dbgF"):
            io[n] = nc.dram_tensor(n, [P, T], BF16, kind="ExternalOutput")
        io["dbgE"] = nc.dram_tensor("dbgE", [HS + 1, T], F32,
                                    kind="ExternalOutput")
        io["dbgG"] = nc.dram_tensor("dbgG", [P, D], BF16,
                                    kind="ExternalOutput")
        io["dbgH"] = nc.dram_tensor("dbgH", [P, H, HS + 1], BF16,
                                    kind="ExternalOutput")
    io["x"] = nc.dram_tensor("x", [T], I32, kind="ExternalInput")
    io["tok_emb"] = nc.dram_tensor("tok_emb", [V, D], F32, kind="ExternalInput")
    io["pos_emb"] = nc.dram_tensor("pos_emb", [CTX, D], F32, kind="ExternalInput")
    for n, shp in [("ln1_g", [L, D]), ("ln1_b", [L, D]),
                   ("bproj", [L, D]),
                   ("ln2_g", [L, D]), ("ln2_b", [L, D]),
                   ("b1", [L, FF]), ("b2", [L, D]),
                   ("lnf_g", [D]), ("lnf_b", [D])]:
        io[n] = nc.dram_tensor(n, shp, F32, kind="ExternalInput")
    # fused weight uploads: QKVP concat on the vocab axis, W1|W2 pre-packed
    # into the on-chip [P, k*n] layout so each is one contiguous DMA
    io["Wqkvp"] = nc.dram_tensor("Wqkvp", [L, D, 4 * D], BF16,
                                 kind="ExternalInput")
    io["W12"] = nc.dram_tensor("W12", [L, P, DT * FF + NT * D], BF16,
                               kind="ExternalInput")
    io["Wout"] = nc.dram_tensor("Wout", [D, V], BF16, kind="ExternalInput")
    io["logits"] = nc.dram_tensor("logits", [T, V], F16, kind="ExternalOutput")

    ident = np.eye(P, dtype=np.float32)
    ntriu = (1.0 - np.triu(np.ones((P, P), np.float32))) * -1e9
    io["consts_d"] = nc.inline_tensor(
        np.concatenate([ident, ntriu], axis=1).astype(BF16_NP),
        name="consts_c")
    io["ones64_d"] = nc.inline_tensor(np.ones((1, HS), np.float32),
                                      name="ones64_c")

    with tile.TileContext(nc) as tc:
        _emit(nc, tc, io, trivial_gb, trivial_bias)
    nc.compile()
    return nc


def _emit(nc, tc, io, trivial_gb, trivial_bias):
    from contextlib import ExitStack
    with ExitStack() as ictx:
        pp_mm = ictx.enter_context(tc.tile_pool(name="pp_mm", bufs=4,
                                                space="PSUM"))
        pp_tp = ictx.enter_context(tc.tile_pool(name="pp_tp", bufs=2,
                                                space="PSUM"))
        pp_at = ictx.enter_context(tc.tile_pool(name="pp_at", bufs=2,
                                                space="PSUM"))
        const = ictx.enter_context(tc.tile_pool(name="const", bufs=1))
        hp = ictx.enter_context(tc.tile_pool(name="hp", bufs=1))
        act = ictx.enter_context(tc.tile_pool(name="act", bufs=6))
        atp = ictx.enter_context(tc.tile_pool(name="atp", bufs=2))
        qkp = ictx.enter_context(tc.tile_pool(name="qkp", bufs=1))
        vp = ictx.enter_context(tc.tile_pool(name="vp", bufs=5))
        ptp = ictx.enter_context(tc.tile_pool(name="ptp", bufs=8))
        up = ictx.enter_context(tc.tile_pool(name="up", bufs=2))
        otp = ictx.enter_context(tc.tile_pool(name="otp", bufs=1))
        gtp = ictx.enter_context(tc.tile_pool(name="gtp", bufs=13))
        wp = ictx.enter_context(tc.tile_pool(name="wp", bufs=2))
        w1p = ictx.enter_context(tc.tile_pool(name="w1p", bufs=2))
        bcp = ictx.enter_context(tc.tile_pool(name="bcp", bufs=4))
        smp = ictx.enter_context(tc.tile_pool(name="smp", bufs=6))
        rcp = ictx.enter_context(tc.tile_pool(name="rcp", bufs=3))
        hfp = ictx.enter_context(tc.tile_pool(name="hfp", bufs=1))
        wop = ictx.enter_context(tc.tile_pool(name="wop", bufs=4))
        lop = ictx.enter_context(tc.tile_pool(name="lop", bufs=3))

        # ---- embedding inputs + constants, fused DMAs (HWDGE is 625ns
        # serial per DMA — fewer, bigger transfers; idx first so the
        # token-embedding gather starts immediately) ----
        epsP = const.tile([P, 1], F32, name="epsP")
        nc.vector.memset(epsP[:], EPS)
        junk = const.tile([1, 1], F32, name="junk")
        nc.vector.memset(junk[:], 1.0)
        it = smp.tile([P, TT], I32, name="idx", tag="idx")
        nc.sync.dma_start(out=it[:],
                          in_=io["x"][:].rearrange("(m p) -> p m", p=P))
        cn = const.tile([P, 2 * P], BF16, name="cn")
        nc.sync.dma_start(out=cn[:], in_=io["consts_d"][:])
        ident = const.tile([P, P], BF16, name="identt")
        nc.vector.tensor_copy(out=ident[:], in_=cn[:, 0:P])
        ntriu = const.tile([P, P], BF16, name="ntriut")
        nc.vector.tensor_copy(out=ntriu[:], in_=cn[:, P:2 * P])
        ident = ident[:]
        ntriu = ntriu[:]
        pos4 = act.tile([P, TT, D], F32, name="pos", tag="pos")
        nc.sync.dma_start(out=pos4[:],
                          in_=io["pos_emb"][0:T, :]
                          .rearrange("(m p) n -> p m n", p=P))
        h_sb = []
        for m in range(TT):
            ht = hp.tile([P, D], F32, name=f"h{m}", tag=f"h{m}")
            nc.gpsimd.indirect_dma_start(
                out=ht[:], out_offset=None, in_=io["tok_emb"][:],
                in_offset=bass.IndirectOffsetOnAxis(ap=it[:, m:m + 1], axis=0))
            h_sb.append(ht)
        for m in range(TT):
            nc.vector.tensor_tensor(out=h_sb[m][:], in0=h_sb[m][:],
                                    in1=pos4[:, m, :], op=ALU.add)
        if DEBUG_TAPS:
            nc.sync.dma_start(out=io["dbg0"][:, 0, :], in_=h_sb[0][:])
        ones64 = const.tile([1, HS], F32R, name="ones64")
        nc.sync.dma_start(out=ones64[:], in_=io["ones64_d"][:].bitcast(F32R))

        # ---- weight prefetch: one fused DMA for QKVP, one for W1|W2 ----
        wts = {}

        def load_qkvp(l):
            # Pool/SWDGE queue: generates AFTER the embedding gather so the
            # startup-critical transfers win the serial DMA unit
            wt = wp.tile([P, DT, 4 * D], BF16, name=f"wqkvp{l}", tag="wdd")
            nc.sync.dma_start(
                out=wt[:],
                in_=io["Wqkvp"][l].rearrange("(k p) n -> p k n", p=P))
            wts[("qkvp", l)] = wt

        def load_w12(l):
            w12 = w1p.tile([P, DT * FF + NT * D], BF16, name=f"w12_{l}",
                           tag="w1")
            nc.sync.dma_start(out=w12[:], in_=io["W12"][l])
            wts[("w12", l)] = w12

        def layer_norm(src_tiles, g_dram, b_dram, tag, defer_scale=False):
            """LN with the apply on ACT (per-partition scale/bias).

            defer_scale: output is only (x - mean); the 1/std factor is
            returned per tile to be folded into a later per-token multiply
            (exact for FF/head since relu is positively homogeneous and the
            rest is linear). Only used when gains/biases are trivial.
            """
            defer = defer_scale and trivial_gb
            if not trivial_gb:
                g_bc = bcp.tile([P, D], F32, name=f"g_{tag}", tag="gb")
                nc.sync.dma_start(out=g_bc[:],
                                  in_=g_dram[None, :].to_broadcast([P, D]))
                b_bc = bcp.tile([P, D], F32, name=f"b_{tag}", tag="gb")
                nc.sync.dma_start(out=b_bc[:],
                                  in_=b_dram[None, :].to_broadcast([P, D]))
            outs, rsigs = [], []
            for m in range(TT):
                st = smp.tile([P, 6], F32, name=f"st_{tag}{m}", tag="st")
                nc.vector.bn_stats(out=st[:], in_=src_tiles[m][:])
                mv = smp.tile([P, 2], F32, name=f"mv_{tag}{m}", tag="mv")
                nc.vector.bn_aggr(out=mv[:], in_=st[:])
                sc = smp.tile([P, 2], F32, name=f"sc_{tag}{m}", tag="sc")
                at = act.tile([P, D], BF16, name=f"a_{tag}{m}", tag="af")
                with nc.allow_low_precision(reason="bf16 matmul operands"):
                    if defer:
                        # critical path: just subtract the mean on ACT
                        nc.vector.tensor_scalar(out=sc[:, 0:1],
                                                in0=mv[:, 0:1],
                                                scalar1=-1.0, scalar2=None,
                                                op0=ALU.mult)
                        nc.scalar.activation(out=at[:], in_=src_tiles[m][:],
                                             func=AF.Identity, bias=sc[:, 0:1])
                        # 1/std off the critical path, consumed later
                        nc.scalar.activation(out=sc[:, 1:2], in_=mv[:, 1:2],
                                             func=AF.Sqrt, bias=epsP[:])
                        nc.vector.reciprocal(out=sc[:, 1:2], in_=sc[:, 1:2])
                    else:
                        nc.scalar.activation(out=mv[:, 1:2], in_=mv[:, 1:2],
                                             func=AF.Sqrt, bias=epsP[:])
                        nc.vector.reciprocal(out=mv[:, 1:2], in_=mv[:, 1:2])
                        # -mean/std so the apply is one ACT scale+bias op
                        nc.vector.tensor_scalar(out=sc[:, 0:1],
                                                in0=mv[:, 0:1],
                                                scalar1=mv[:, 1:2],
                                                scalar2=-1.0,
                                                op0=ALU.mult, op1=ALU.mult)
                        nc.scalar.activation(out=at[:], in_=src_tiles[m][:],
                                             func=AF.Identity, scale=mv[:, 1:2],
                                             bias=sc[:, 0:1])
                        if not trivial_gb:
                            nc.vector.tensor_tensor(out=at[:], in0=at[:],
                                                    in1=g_bc[:], op=ALU.mult)
                            nc.vector.tensor_tensor(out=at[:], in0=at[:],
                                                    in1=b_bc[:], op=ALU.add)
                outs.append(at)
                rsigs.append(sc[:, 1:2] if defer else mv[:, 1:2])
            return outs, rsigs

        def transpose_to(a_tiles, tag, pool):
            """[TT][P,D] natural bf16 -> [DT][P,T] bf16 transposed;
            evictions alternate ACT/DVE so the two PSUM bufs free fast."""
            outs = []
            for k in range(DT):
                ps = pp_tp.tile([P, T], BF16, name=f"tp_{tag}{k}", tag="tp")
                for m in range(TT):
                    nc.tensor.transpose(ps[:, P * m:P * (m + 1)],
                                        a_tiles[m][:, P * k:P * (k + 1)],
                                        ident[:])
                dst = pool.tile([P, T], BF16, name=f"{tag}T{k}", tag=f"aT{k}")
                ev_act(dst, ps)  # NB: DVE misreads bf16-in-PSUM layouts
                outs.append(dst)
            return outs

        def ev_act(dst, ps):
            nc.scalar.copy(dst[:], ps[:])

        def ev_dve(dst, ps):
            nc.vector.tensor_copy(out=dst[:], in_=ps[:])

        if DEBUG_TAPS:
            def tap(i, ap):
                if i < 8:
                    nc.sync.dma_start(out=io[f"dbg{i}"][:], in_=ap)
        else:
            def tap(i, ap):
                pass


        # ================= layers =================
        load_qkvp(0)
        for l in range(L):
            load_w12(l)
            if l + 1 < L:
                load_qkvp(l + 1)
            wqkvp = wts.pop(("qkvp", l))
            w12 = wts.pop(("w12", l))

            def wqkv(which, d, c0, w):
                return wqkvp[:, d, which * D + c0:which * D + c0 + w]

            def w1s(d, c0, w):
                return w12[:, d * FF + c0:d * FF + c0 + w]

            def w2s(nt):
                return w12[:, DT * FF + nt * D:DT * FF + (nt + 1) * D]
            if not trivial_bias:
                b1c = smp.tile([P, NT], F32, name=f"b1c{l}", tag="b1c")
                nc.sync.dma_start(
                    out=b1c[:],
                    in_=io["b1"][l, :].rearrange("(n p) -> p n", p=P))

            a_t, _ = layer_norm(h_sb, io["ln1_g"][l, :], io["ln1_b"][l, :],
                                f"l{l}n1")
            if DEBUG_TAPS and l == 0:
                dga = const.tile([P, D], BF16, name="dga")
                nc.vector.tensor_copy(out=dga[:], in_=a_t[0][:])
                nc.sync.dma_start(out=io["dbgG"][:], in_=dga[:])
            aT = transpose_to(a_t, f"l{l}a", atp)
            if DEBUG_TAPS and l == 0:
                dgb = const.tile([P, T], BF16, name="dgb")
                nc.vector.tensor_copy(out=dgb[:], in_=aT[0][:])
                nc.sync.dma_start(out=io["dbgA"][:], in_=dgb[:])


            v_sb = []
            for m in range(TT):
                ps = pp_mm.tile([P, D], F32, name=f"ps_v{l}{m}", tag="mm")
                for d in range(DT):
                    nc.tensor.matmul(ps[:], aT[d][:, P * m:P * (m + 1)],
                                     wqkv(2, d, 0, D), start=(d == 0),
                                     stop=(d == DT - 1))
                vt = vp.tile([P, H, HS + 1], BF16, name=f"v{l}{m}", tag="v")
                nc.vector.tensor_copy(out=vt[:, :, 0:HS],
                                      in_=ps[:].rearrange("p (h d) -> p h d",
                                                          h=H))
                nc.gpsimd.memset(vt[:, :, HS], 1.0)
                if DEBUG_TAPS and l == 0 and m == 0:
                    dgv = const.tile([P, H, HS + 1], BF16, name="dgv")
                    nc.vector.tensor_copy(out=dgv[:], in_=vt[:])
                    nc.sync.dma_start(out=io["dbgH"][:], in_=dgv[:])
                v_sb.append(vt)
            qT, kT = [], []
            for which, name, dstl, ev in ((0, "q", qT, ev_act),
                                          (1, "k", kT, ev_dve)):
                for dq in range(DT):
                    ps = pp_mm.tile([P, T], F32, name=f"ps_{name}{l}{dq}",
                                    tag="mm")
                    for d in range(DT):
                        nc.tensor.matmul(ps[:], wqkv(which, d, P * dq, P),
                                         aT[d][:], start=(d == 0),
                                         stop=(d == DT - 1))
                    dst = qkp.tile([P, T], BF16, name=f"{name}T{l}{dq}",
                                   tag=f"{name}T{dq}")
                    ev(dst, ps)
                    if DEBUG_TAPS and l == 0 and dq == 0:
                        dgq = const.tile([P, T], BF16, name=f"dgq{which}")
                        nc.vector.tensor_copy(out=dgq[:], in_=dst[:])
                        nc.sync.dma_start(
                            out=io["dbgB" if which == 0 else "dbgC"][:],
                            in_=dgq[:])
                    dstl.append(dst)

            # attention, 2 heads (one oT row-block) at a time
            oT = [otp.tile([P, T], BF16, name=f"oT{l}{k}", tag=f"oT{k}")
                  for k in range(DT)]
            for r in range(DT):
                upss = []
                for hh in range(2):
                    h = 2 * r + hh
                    off = hh * HS
                    pT = []
                    for j in range(TT):
                        n_j = T - P * j
                        ps = pp_mm.tile([P, T], F32, name=f"ps_s{l}{h}{j}",
                                        tag="mm")
                        nc.tensor.matmul(
                            ps[:, 0:n_j],
                            kT[r][off:off + HS, P * j:P * (j + 1)],
                            qT[r][off:off + HS, P * j:T],
                            start=True, stop=False, skip_group_check=True)
                        # causal mask: -1e9 upper triangle accumulated onto
                        # the diagonal block by the PE (not a DVE pass)
                        nc.tensor.matmul(
                            ps[:, 0:P], ident[:], ntriu[:],
                            start=False, stop=True, skip_group_check=True)
                        pt = ptp.tile([P, T], BF16, name=f"pT{l}{h}{j}",
                                      tag="pT")
                        nc.scalar.activation(out=pt[:, 0:n_j], in_=ps[:, 0:n_j],
                                             func=AF.Exp, scale=SCALE)
                        pT.append(pt)
                    ups = pp_at.tile([HS + 1, T], F32, name=f"ups{l}{h}",
                                     tag="at")
                    for j in range(TT):
                        n_j = T - P * j
                        nc.tensor.matmul(ups[:, P * j:T], v_sb[j][:, h, :],
                                         pT[j][:, 0:n_j],
                                         start=(j == 0), stop=(j == TT - 1))
                    upss.append(ups)
                with nc.allow_low_precision(reason="softmax denom/attn bf16"):
                    for hh in range(2):
                        off = hh * HS
                        u = up.tile([HS, T], BF16, name=f"u{l}{r}{hh}",
                                    tag="u")
                        nc.vector.tensor_copy(out=u[:],
                                              in_=upss[hh][0:HS, :])
                        rec = rcp.tile([1, T], F32R, name=f"rec{l}{r}{hh}",
                                       tag="rec")
                        nc.vector.reciprocal(out=rec[:],
                                             in_=upss[hh][HS:HS + 1, :])
                        bc = pp_at.tile([HS, T], F32, name=f"bc{l}{r}{hh}",
                                        tag="at")
                        nc.tensor.matmul(bc[:], ones64[:], rec[:], start=True,
                                         stop=True)
                        nc.vector.tensor_tensor(out=oT[r][off:off + HS, :],
                                                in0=u[:], in1=bc[:],
                                                op=ALU.mult)

            if DEBUG_TAPS and l == 0:
                nc.sync.dma_start(out=io["dbgF"][:], in_=oT[0][:])
            # proj + residual (natural out)
            if not trivial_bias:
                bp_bc = bcp.tile([P, D], F32, name=f"bp{l}", tag="gb")
                nc.sync.dma_start(
                    out=bp_bc[:],
                    in_=io["bproj"][l, None, :].to_broadcast([P, D]))
            for m in range(TT):
                ps = pp_mm.tile([P, D], F32, name=f"ps_pj{l}{m}", tag="mm")
                for d in range(DT):
                    nc.tensor.matmul(ps[:], oT[d][:, P * m:P * (m + 1)],
                                     wqkv(3, d, 0, D), start=(d == 0),
                                     stop=(d == DT - 1))
                if not trivial_bias:
                    nc.vector.tensor_tensor(out=ps[:], in0=ps[:],
                                            in1=bp_bc[:], op=ALU.add)
                nc.vector.tensor_tensor(out=h_sb[m][:], in0=h_sb[m][:],
                                        in1=ps[:], op=ALU.add)


            if DEBUG_TAPS and l == 0:
                nc.sync.dma_start(out=io["dbg1"][:, 0, :], in_=h_sb[0][:])
            # ---- FF ----
            defer2 = trivial_bias  # rsig folding needs b1 == 0
            f_t, rsig2 = layer_norm(h_sb, io["ln2_g"][l, :], io["ln2_b"][l, :],
                                    f"l{l}n2", defer_scale=defer2)
            fT = transpose_to(f_t, f"l{l}f", atp)

            gts = []
            for nt in range(NT):
                psg = pp_tp.tile([P, T], F32, name=f"ps_g{l}{nt}", tag="tp")
                for d in range(DT):
                    nc.tensor.matmul(psg[:], w1s(d, P * nt, P),
                                     fT[d][:], start=(d == 0),
                                     stop=(d == DT - 1))
                gt = gtp.tile([P, T], BF16, name=f"g{l}{nt}", tag="g")
                if trivial_bias:
                    nc.scalar.activation(out=gt[:], in_=psg[:], func=AF.Relu)
                else:
                    nc.scalar.activation(out=gt[:], in_=psg[:], func=AF.Relu,
                                         bias=b1c[:, nt:nt + 1])
                gts.append(gt)
            if not trivial_bias:
                b2_bc = bcp.tile([P, D], F32, name=f"b2{l}", tag="gb")
                nc.sync.dma_start(
                    out=b2_bc[:],
                    in_=io["b2"][l, None, :].to_broadcast([P, D]))
            # m-outer: each token tile's FF output completes early so its
            # residual + next LN pipeline behind the remaining matmuls
            for m in range(TT):
                ps = pp_mm.tile([P, D], F32, name=f"ps_ff{l}{m}", tag="mm")
                for nt in range(NT):
                    nc.tensor.matmul(ps[:], gts[nt][:, P * m:P * (m + 1)],
                                     w2s(nt), start=(nt == 0),
                                     stop=(nt == NT - 1))
                if defer2:
                    # deferred 1/std of ln2, folded into the residual add
                    nc.vector.scalar_tensor_tensor(
                        out=h_sb[m][:], in0=ps[:], scalar=rsig2[m],
                        in1=h_sb[m][:], op0=ALU.mult, op1=ALU.add)
                else:
                    if not trivial_bias:
                        nc.vector.tensor_tensor(out=ps[:], in0=ps[:],
                                                in1=b2_bc[:], op=ALU.add)
                    nc.vector.tensor_tensor(out=h_sb[m][:], in0=h_sb[m][:],
                                            in1=ps[:], op=ALU.add)


            if DEBUG_TAPS and l == 0:
                nc.sync.dma_start(out=io["dbg2"][:, 0, :], in_=h_sb[0][:])
        # ---- final LN -> transposed, stays in SBUF for the head ----
        hf_t, rsigf = layer_norm(h_sb, io["lnf_g"][:], io["lnf_b"][:], "lnf",
                                 defer_scale=True)
        hfT = transpose_to(hf_t, "hf", hfp)

        # ================= token-sharded full-vocab head =================
        wo_tiles = {}

        def load_wo(vc):
            wo = wop.tile([P, DT, VC], BF16, name=f"wo{vc}", tag="wo")
            nc.sync.dma_start(
                out=wo[:],
                in_=io["Wout"][:, vc * VC:(vc + 1) * VC]
                .rearrange("(k p) n -> p k n", p=P))
            wo_tiles[vc] = wo

        for vc in range(min(4, NVC)):
            load_wo(vc)

        ps_pools = [pp_mm] * 4 + [pp_tp] * 2 + [pp_at] * 2
        ps_tags = ["mm"] * 4 + ["tp"] * 2 + ["at"] * 2
        # eviction engine rotation: ACT and DVE alternate
        for vc in range(NVC):
            wo = wo_tiles.pop(vc)
            if vc + 4 < NVC:
                load_wo(vc + 4)
            lo = lop.tile([P, TT, VC], F16, name=f"lo{vc}", tag="lo")
            for m in range(TT):
                for sub in range(NSUB):
                    k8 = (vc * TT * NSUB + m * NSUB + sub) % 8
                    ps = ps_pools[k8].tile([P, SUB], F32,
                                           name=f"ps_o{vc}{m}{sub}",
                                           tag=ps_tags[k8])
                    c0 = sub * SUB
                    for d in range(DT):
                        nc.tensor.matmul(ps[:], hfT[d][:, P * m:P * (m + 1)],
                                         wo[:, d, c0:c0 + SUB],
                                         start=(d == 0), stop=(d == DT - 1))
                    sl = lo[:, m, c0:c0 + SUB]
                    with nc.allow_low_precision(reason="fp16 logits out"):
                        if trivial_gb:
                            # deferred lnf 1/std folded into the eviction
                            if (m + sub) % 2 == 0:
                                nc.scalar.activation(out=sl, in_=ps[:],
                                                     func=AF.Identity,
                                                     scale=rsigf[m])
                            else:
                                nc.vector.tensor_scalar(out=sl, in0=ps[:],
                                                        scalar1=rsigf[m],
                                                        scalar2=None,
                                                        op0=ALU.mult)
                        else:
                            if (m + sub) % 2 == 0:
                                nc.scalar.copy(sl, ps[:])
                            else:
                                nc.vector.tensor_copy(out=sl, in_=ps[:])
            nc.scalar.dma_start(
                out=io["logits"][:, vc * VC:(vc + 1) * VC]
                .rearrange("(m p) n -> p m n", p=P),
                in_=lo[:])


_NC_CACHE = {}


def _get_nc(trivial_gb=False, trivial_bias=False):
    key = (trivial_gb, trivial_bias)
    if key not in _NC_CACHE:
        _NC_CACHE[key] = _build(*key)
    return _NC_CACHE[key]


def _build_sim():
    return _build(trivial_gb=True, trivial_bias=True)


def kernel(**inputs):
    inp = {k: np.ascontiguousarray(np.asarray(v)) for k, v in inputs.items()}
    trivial_gb = all(
        np.all(inp[g] == 1.0) and np.all(inp[b] == 0.0)
        for g, b in [("ln1_g", "ln1_b"), ("ln2_g", "ln2_b"), ("lnf_g", "lnf_b")])
    trivial_bias = all(np.all(inp[b] == 0.0) for b in ("bproj", "b1", "b2"))
    nc = _get_nc(trivial_gb, trivial_bias)
    wqkvp = np.concatenate(
        [inp["Wq"], inp["Wk"], inp["Wv"], inp["Wproj"]],
        axis=2).astype(BF16_NP)                              # [L, D, 4D]
    w1pk = (inp["W1"].astype(BF16_NP)
            .reshape(L, DT, P, FF).transpose(0, 2, 1, 3)
            .reshape(L, P, DT * FF))                         # [L, P, DT*FF]
    w2pk = (inp["W2"].astype(BF16_NP)
            .reshape(L, NT, P, D).transpose(0, 2, 1, 3)
            .reshape(L, P, NT * D))                          # [L, P, NT*D]
    w12 = np.ascontiguousarray(np.concatenate([w1pk, w2pk], axis=2))
    bw = {"Wqkvp": np.ascontiguousarray(wqkvp), "W12": w12,
          "Wout": inp["Wout"].astype(BF16_NP)}
    in_maps = []
    for c in range(N_CORES):
        m = {
            "x": inp["x"][c].astype(np.int32),
            "tok_emb": inp["tok_emb"], "pos_emb": inp["pos_emb"],
            "ln1_g": inp["ln1_g"], "ln1_b": inp["ln1_b"],
            "bproj": inp["bproj"],
            "ln2_g": inp["ln2_g"], "ln2_b": inp["ln2_b"],
            "b1": inp["b1"], "b2": inp["b2"],
            "lnf_g": inp["lnf_g"], "lnf_b": inp["lnf_b"],
            **bw,
        }
        in_maps.append(m)
    res = run_bass_kernel_spmd(nc, in_maps, core_ids=list(range(N_CORES)))
    out = np.stack([res.results[c]["logits"].astype(np.float32)
                    for c in range(N_CORES)], axis=0)
    if np.any(inp["bout"] != 0.0):
        out = out + inp["bout"][None, None, :].astype(np.float32)
    return out
